# revision 3
# baseline (speedup 1.0000x reference)
"""Distributed Trainium2 kernel for nn_AttentionHead (B=8, N=2048, H=E=1024).

Single attention head, causal mask keeping j >= i, softmax over j, per batch:

    K = X Wk; Q = X Wq; V = X Wv
    S = Q K^T / sqrt(E);  S[i, j] = -inf for i > j
    O = softmax_j(S) V

Sharding: pure data parallel - batch b (8) maps 1:1 onto the 8 NeuronCores.
Weights replicated; no collectives.

Per-core algorithm (matmuls bf16, fp32 PSUM):
  Phase 1 (streaming):
    - DMA X tiles with wv chunks interleaved, then wq, wk (one SP queue).
    - X/Wq/Wk cast bf16 (ScalarE), PE-transposed; 4 transposes share one
      PSUM bank and drain with a single DVE copy (DVE instruction count
      is the phase-1 drag otherwise).
    - V = X Wv per row tile, interleaved with the X stream; Wq/Wk chunks
      interleaved into the V tail so their DMA/cast/transpose overlaps PE.
    - A = Wq Wk^T, then GT[h2,i] = sum_h1 A[h1,h2] XT[h1,i]:
      S = Q K^T = X (Wq Wk^T) X^T needs only this one projection.
  Phase 2 (attention, row-block PAIRS, ascending):
    - Scores TRANSPOSED per unit (pair p, j-tile jt): ST[j, i0:i0+256]
      covering both blocks of the pair; 256-wide rhs streams amortize
      weight loads and give exp 2x the hiding window.
    - Causal mask applied INSIDE the score matmul group: one extra
      matmul with lhsT=identity, rhs=precomputed NEG upper-strict tile
      adds -1e30 to masked entries - no cross-engine mask hop.
    - exp on ScalarE writes PT[j,i] bf16 straight to SBUF: PT is the
      ready-made stationary operand for PV (no P transposes).
    - Row sums via 1-column ones-matmuls sharing PV's loaded weights.
    - PSUM banks: 2 x ST + 4 x O (pair) + 2 x rowsum = 8.
    - O rows scaled by 1/rowsum (ScalarE), DMA out per 512 columns.
"""

import numpy as np

try:
    import concourse.bass as bass
except ImportError:  # fresh grading dir: concourse comes from the site repo
    import sys

    for p in ("/opt/trn_rl_repo", "/root/.axon_site/_ro/trn_rl_repo"):
        if p not in sys.path:
            sys.path.append(p)
    import concourse.bass as bass

import concourse.mybir as mybir
import concourse.tile as tile
from concourse import bacc, bass_utils
from concourse.masks import make_identity

B, N, H, E = 8, 2048, 1024, 1024
P = 128
HT = H // P  # 8 h-tiles
ET = E // P  # 8 e-tiles
NT = N // P  # 16 row tiles
F32 = mybir.dt.float32
BF16 = mybir.dt.bfloat16
SCALE = 1.0 / float(np.sqrt(E))
NEG = -1.0e30


def build_graph():
    nc = bacc.Bacc("TRN2", target_bir_lowering=False, debug=False,
                   enable_asserts=False)
    x = nc.dram_tensor("input", [N, H], F32, kind="ExternalInput").ap()
    wk = nc.dram_tensor("k", [H, E], F32, kind="ExternalInput").ap()
    wq = nc.dram_tensor("q", [H, E], F32, kind="ExternalInput").ap()
    wv = nc.dram_tensor("v", [H, E], F32, kind="ExternalInput").ap()
    out = nc.dram_tensor("out", [N, E], F32, kind="ExternalOutput").ap()

    with tile.TileContext(nc) as tc:
        with (
            tc.tile_pool(name="const", bufs=1) as constp,
            tc.tile_pool(name="persist", bufs=1) as persist,
        ):
            ident16 = constp.tile([P, P], BF16)
            make_identity(nc, ident16)
            # maskR[p, i] = NEG where p < i else 0. Matmul with
            # lhsT=ident16 reproduces it into ST[j, i]: adds NEG to the
            # strictly-masked (j < i) entries of a diagonal unit.
            maskR = constp.tile([P, P], BF16)
            nc.gpsimd.memset(maskR, 0.0)
            # keep 0 where iota = p - i >= 0, fill NEG where p < i
            # (is_ge is the only comparison the neuronx backend implements
            # for affine_select)
            nc.gpsimd.affine_select(
                out=maskR, in_=maskR, compare_op=mybir.AluOpType.is_ge,
                fill=NEG, base=0, pattern=[[-1, P]], channel_multiplier=1,
            )
            ones16 = constp.tile([P, 1], BF16)
            nc.gpsimd.memset(ones16, 1.0)
            # touch Exp once so the activation table is resident before
            # the first real exp on the phase-2 critical path
            warm = constp.tile([P, 1], F32)
            nc.scalar.activation(
                warm, ones16, mybir.ActivationFunctionType.Exp,
                bias=0.0, scale=1.0,
            )

            xt = persist.tile([P, HT, N], BF16)  # X^T [h, i] (K-side operand)
            gt = persist.tile([P, HT, N], BF16)  # G^T [h2, i] (Q-side)
            vt = persist.tile([P, NT, E], BF16)  # V   [j, e]

            with (
                tc.tile_pool(name="ph1", bufs=1) as ph1,
                tc.tile_pool(name="stage", bufs=3) as stage,
                tc.tile_pool(name="psA", bufs=5, space="PSUM") as psA,
                tc.tile_pool(name="psT", bufs=3, space="PSUM") as psT,
            ):
                wvb = ph1.tile([P, HT, E], BF16, tag="wv")  # Wv natural [h, e]
                wqT = ph1.tile([P, ET, H], BF16, tag="wqT")  # Wq^T [e, h1]
                wkT = ph1.tile([P, ET, H], BF16, tag="wkT")  # Wk^T [e, h2]
                ab = ph1.tile([P, HT, H], BF16, tag="A")  # A [h1, h2]

                def emit_wv(ho):
                    ws = stage.tile([P, E], F32, tag="wst")
                    nc.sync.dma_start(ws, wv[ho * P:(ho + 1) * P, :])
                    nc.vector.tensor_copy(wvb[:, ho, :], ws)

                def emit_x_tile(it):
                    for hh in range(2):
                        xs = stage.tile([P, H // 2], F32, tag="xst")
                        nc.sync.dma_start(
                            xs, x[it * P:(it + 1) * P,
                                  hh * (H // 2):(hh + 1) * (H // 2)])
                        xb = stage.tile([P, H // 2], BF16, tag="xbt")
                        nc.scalar.copy(xb, xs)
                        tp4 = psT.tile([P, 4, P], BF16, tag="tp")
                        for hi in range(HT // 2):
                            nc.tensor.transpose(
                                tp4[:, hi, :], xb[:, hi * P:(hi + 1) * P],
                                ident16)
                        nc.vector.tensor_copy(
                            xt[:, 4 * hh:4 * (hh + 1), it * P:(it + 1) * P],
                            tp4)

                def emit_w_chunk(wsrc, wdstT, ho):
                    """One [128, 1024] row chunk of wq/wk: DMA, cast, 8 PE
                    transposes (batched 4 per PSUM bank + single copy)."""
                    ws = stage.tile([P, E], F32, tag="wst")
                    nc.sync.dma_start(ws, wsrc[ho * P:(ho + 1) * P, :])
                    wb = stage.tile([P, E], BF16, tag="wbt")
                    nc.scalar.copy(wb, ws)
                    for eh in range(2):
                        tp4 = psT.tile([P, 4, P], BF16, tag="tp")
                        for ei in range(4):
                            nc.tensor.transpose(
                                tp4[:, ei, :],
                                wb[:, (4 * eh + ei) * P:(4 * eh + ei + 1) * P],
                                ident16)
                        nc.vector.tensor_copy(
                            wdstT[:, 4 * eh:4 * (eh + 1),
                                  ho * P:(ho + 1) * P], tp4)

                def emit_v_proj(jt):
                    for es in range(E // 512):
                        mm = psA.tile([P, 512], F32, tag="mm")
                        for ho in range(HT):
                            nc.tensor.matmul(
                                mm,
                                lhsT=xt[:, ho, jt * P:(jt + 1) * P],
                                rhs=wvb[:, ho, es * 512:(es + 1) * 512],
                                start=(ho == 0), stop=(ho == HT - 1),
                            )
                        nc.vector.tensor_copy(
                            vt[:, jt, es * 512:(es + 1) * 512], mm)

                # DMA order: x0 first (transposes start immediately), then
                # all wv (V(0)'s matmuls dribble in per arriving chunk - the
                # per-operand deps handle it), then the remaining X tiles
                # with V-projection lagging one tile, wq/wk streaming into
                # the back half.
                wlist = [(wq, wqT, ho) for ho in range(HT)] + \
                        [(wk, wkT, ho) for ho in range(HT)]
                emit_x_tile(0)
                emit_x_tile(1)
                for ho in range(HT):
                    emit_wv(ho)
                emit_v_proj(0)
                emit_v_proj(1)
                for it in range(2, NT):
                    emit_x_tile(it)
                    if it >= 3:
                        emit_v_proj(it - 1)
                    if it >= 8:
                        for c in (2 * (it - 8), 2 * (it - 8) + 1):
                            emit_w_chunk(*wlist[c])
                emit_v_proj(NT - 1)

                # ---- A = Wq Wk^T : A[h1, h2] = sum_e WqT[e,h1] WkT[e,h2]
                for h1t in range(HT):
                    for h2s in range(H // 512):
                        mm = psA.tile([P, 512], F32, tag="mm")
                        for et in range(ET):
                            nc.tensor.matmul(
                                mm,
                                lhsT=wqT[:, et, h1t * P:(h1t + 1) * P],
                                rhs=wkT[:, et, h2s * 512:(h2s + 1) * 512],
                                start=(et == 0), stop=(et == ET - 1),
                            )
                        nc.vector.tensor_copy(
                            ab[:, h1t, h2s * 512:(h2s + 1) * 512], mm)

                # ---- GT[h2, i] = sum_h1 A[h1, h2] XT[h1, i] ----
                for ns in range(N // 512):
                    for h2t in range(HT):
                        mm = psA.tile([P, 512], F32, tag="mm")
                        for h1t in range(HT):
                            nc.tensor.matmul(
                                mm,
                                lhsT=ab[:, h1t, h2t * P:(h2t + 1) * P],
                                rhs=xt[:, h1t, ns * 512:(ns + 1) * 512],
                                start=(h1t == 0), stop=(h1t == HT - 1),
                            )
                        nc.vector.tensor_copy(
                            gt[:, h2t, ns * 512:(ns + 1) * 512], mm)

            # ---- attention: row-block pairs (it0, it0+1), ascending ----
            with (
                tc.tile_pool(name="work", bufs=4) as work,
                tc.tile_pool(name="obuf", bufs=4) as obuf,
                tc.tile_pool(name="accp", bufs=2) as accp,
                tc.tile_pool(name="psS", bufs=2, space="PSUM") as psS,
                tc.tile_pool(name="psO", bufs=4, space="PSUM") as psO,
                tc.tile_pool(name="psR", bufs=2, space="PSUM") as psR,
            ):
                for pr in range(NT // 2):
                    it0 = 2 * pr
                    it1 = it0 + 1
                    i0 = it0 * P

                    def emit_st(jt, it0=it0, i0=i0):
                        """ST[j, i0:i0+w] for unit (pair, jt). First unit
                        covers only the low block (high block fully masked).
                        Mask lands as an extra matmul inside the group."""
                        w = P if jt == it0 else 2 * P
                        diag = jt in (it0, it0 + 1)
                        sp = psS.tile([P, 2 * P], F32, tag="s")
                        for et in range(ET):
                            nc.tensor.matmul(
                                sp[:, :w],
                                lhsT=xt[:, et, jt * P:(jt + 1) * P],
                                rhs=gt[:, et, i0:i0 + w],
                                start=(et == 0),
                                stop=(et == ET - 1) and not diag,
                            )
                        if diag:
                            off = 0 if jt == it0 else P
                            nc.tensor.matmul(
                                sp[:, off:off + P], lhsT=ident16, rhs=maskR,
                                start=False, stop=True,
                            )
                        pb = work.tile([P, 2 * P], BF16, tag="p")
                        nc.scalar.activation(
                            pb[:, :w], sp[:, :w],
                            mybir.ActivationFunctionType.Exp,
                            bias=0.0, scale=SCALE,
                        )
                        return pb

                    ol0 = psO.tile([P, 512], F32, tag="o")
                    ol1 = psO.tile([P, 512], F32, tag="o")
                    oh0 = psO.tile([P, 512], F32, tag="o")
                    oh1 = psO.tile([P, 512], F32, tag="o")
                    rsl = psR.tile([P, 1], F32, tag="rs")
                    rsh = psR.tile([P, 1], F32, tag="rs")

                    pb_prev = emit_st(it0)
                    for jt in range(it0, NT):
                        pb = pb_prev
                        if jt + 1 < NT:
                            pb_prev = emit_st(jt + 1)
                        lo_f, hi_f = jt == it0, jt == it1
                        last = jt == NT - 1
                        nc.tensor.matmul(
                            ol0, lhsT=pb[:, 0:P], rhs=vt[:, jt, 0:512],
                            start=lo_f, stop=last)
                        nc.tensor.matmul(
                            ol1, lhsT=pb[:, 0:P], rhs=vt[:, jt, 512:1024],
                            start=lo_f, stop=last)
                        nc.tensor.matmul(
                            rsl, lhsT=pb[:, 0:P], rhs=ones16,
                            start=lo_f, stop=last)
                        if jt > it0:
                            nc.tensor.matmul(
                                oh0, lhsT=pb[:, P:2 * P], rhs=vt[:, jt, 0:512],
                                start=hi_f, stop=last)
                            nc.tensor.matmul(
                                oh1, lhsT=pb[:, P:2 * P],
                                rhs=vt[:, jt, 512:1024],
                                start=hi_f, stop=last)
                            nc.tensor.matmul(
                                rsh, lhsT=pb[:, P:2 * P], rhs=ones16,
                                start=hi_f, stop=last)

                    # scale + drain on DVE (Act stays clear for the next
                    # pair's exp chain); low block first - its PSUM banks
                    # are the ones the next pair needs soonest. Last pair:
                    # split across DVE and Act to shorten the tail.
                    last_pair = pr == NT // 2 - 1
                    for (itx, rsx, ops, use_act) in (
                            (it0, rsl, (ol0, ol1), False),
                            (it1, rsh, (oh0, oh1), last_pair)):
                        ri = accp.tile([P, 1], F32, tag="ri")
                        nc.vector.reciprocal(ri, rsx)
                        for es, op in enumerate(ops):
                            ob = obuf.tile([P, 512], F32, tag="ob")
                            if use_act:
                                nc.scalar.mul(ob, op, ri)
                            else:
                                nc.vector.tensor_scalar_mul(ob, op, ri)
                            # last pair drains via the idle Pool queue
                            # (25ns issue vs SP's 565ns) to shorten the tail
                            dma_eng = nc.gpsimd if last_pair else nc.sync
                            dma_eng.dma_start(
                                out[itx * P:(itx + 1) * P,
                                    es * 512:(es + 1) * 512], ob)

    nc.finalize()
    return nc


_NC = None


def _get_nc():
    global _NC
    if _NC is None:
        _NC = build_graph()
    return _NC


def _run(inputs, trace=False, **kwargs):
    x = np.ascontiguousarray(np.asarray(inputs["input"], dtype=np.float32))
    k = np.ascontiguousarray(np.asarray(inputs["k"], dtype=np.float32))
    q = np.ascontiguousarray(np.asarray(inputs["q"], dtype=np.float32))
    v = np.ascontiguousarray(np.asarray(inputs["v"], dtype=np.float32))
    assert x.shape == (B, N, H)
    nc = _get_nc()
    in_maps = [
        {"input": x[b], "k": k, "q": q, "v": v} for b in range(B)
    ]
    res = bass_utils.run_bass_kernel_spmd(
        nc, in_maps, core_ids=list(range(B)), trace=trace, **kwargs)
    outs = np.stack([np.asarray(r["out"]) for r in res.results], axis=0)
    return outs.astype(np.float32), res


def kernel(**inputs):
    outs, _ = _run(inputs, trace=False)
    return outs


# revision 5
# speedup vs baseline: 1.0011x; 1.0011x over previous
"""Distributed Trainium2 kernel for nn_AttentionHead (B=8, N=2048, H=E=1024).

Single attention head, causal mask keeping j >= i, softmax over j, per batch:

    K = X Wk; Q = X Wq; V = X Wv
    S = Q K^T / sqrt(E);  S[i, j] = -inf for i > j
    O = softmax_j(S) V

Sharding: pure data parallel - batch b (8) maps 1:1 onto the 8 NeuronCores.
Weights replicated; no collectives.

Per-core algorithm (matmuls bf16, fp32 PSUM):
  Phase 1 (streaming):
    - DMA X tiles with wv chunks interleaved, then wq, wk (one SP queue).
    - X/Wq/Wk cast bf16 (ScalarE), PE-transposed; 4 transposes share one
      PSUM bank and drain with a single DVE copy (DVE instruction count
      is the phase-1 drag otherwise).
    - V = X Wv per row tile, interleaved with the X stream; Wq/Wk chunks
      interleaved into the V tail so their DMA/cast/transpose overlaps PE.
    - A = Wq Wk^T, then GT[h2,i] = sum_h1 A[h1,h2] XT[h1,i]:
      S = Q K^T = X (Wq Wk^T) X^T needs only this one projection.
  Phase 2 (attention, row-block PAIRS ascending, last 2 blocks single):
    - Scores TRANSPOSED per unit (pair p, j-tile jt): ST[j, i0:i0+256]
      covering both blocks of the pair; 256-wide rhs streams amortize
      weight loads and give exp 2x the hiding window.
    - Causal mask applied INSIDE the score matmul group: one extra
      matmul with lhsT=identity, rhs=precomputed NEG upper-strict tile
      adds -1e30 to masked entries - no cross-engine mask hop.
    - exp on ScalarE writes PT[j,i] bf16 straight to SBUF: PT is the
      ready-made stationary operand for PV (no P transposes).
    - Row sums via 1-column ones-matmuls sharing PV's loaded weights.
    - PSUM banks: 2 x ST + 4 x O (pair) + 2 x rowsum = 8.
    - O rows scaled by 1/rowsum (ScalarE), DMA out per 512 columns.
"""

import numpy as np

try:
    import concourse.bass as bass
except ImportError:  # fresh grading dir: concourse comes from the site repo
    import sys

    for p in ("/opt/trn_rl_repo", "/root/.axon_site/_ro/trn_rl_repo"):
        if p not in sys.path:
            sys.path.append(p)
    import concourse.bass as bass

import concourse.mybir as mybir
import concourse.tile as tile
from concourse import bacc, bass_utils
from concourse.masks import make_identity

B, N, H, E = 8, 2048, 1024, 1024
P = 128
HT = H // P  # 8 h-tiles
ET = E // P  # 8 e-tiles
NT = N // P  # 16 row tiles
F32 = mybir.dt.float32
BF16 = mybir.dt.bfloat16
SCALE = 1.0 / float(np.sqrt(E))
NEG = -1.0e30


def build_graph():
    nc = bacc.Bacc("TRN2", target_bir_lowering=False, debug=False,
                   enable_asserts=False)
    x = nc.dram_tensor("input", [N, H], F32, kind="ExternalInput").ap()
    wk = nc.dram_tensor("k", [H, E], F32, kind="ExternalInput").ap()
    wq = nc.dram_tensor("q", [H, E], F32, kind="ExternalInput").ap()
    wv = nc.dram_tensor("v", [H, E], F32, kind="ExternalInput").ap()
    out = nc.dram_tensor("out", [N, E], F32, kind="ExternalOutput").ap()

    with tile.TileContext(nc) as tc:
        with (
            tc.tile_pool(name="const", bufs=1) as constp,
            tc.tile_pool(name="persist", bufs=1) as persist,
        ):
            ident16 = constp.tile([P, P], BF16)
            make_identity(nc, ident16)
            # maskR[p, i] = NEG where p < i else 0. Matmul with
            # lhsT=ident16 reproduces it into ST[j, i]: adds NEG to the
            # strictly-masked (j < i) entries of a diagonal unit.
            maskR = constp.tile([P, P], BF16)
            nc.gpsimd.memset(maskR, 0.0)
            # keep 0 where iota = p - i >= 0, fill NEG where p < i
            # (is_ge is the only comparison the neuronx backend implements
            # for affine_select)
            nc.gpsimd.affine_select(
                out=maskR, in_=maskR, compare_op=mybir.AluOpType.is_ge,
                fill=NEG, base=0, pattern=[[-1, P]], channel_multiplier=1,
            )
            ones16 = constp.tile([P, 1], BF16)
            nc.gpsimd.memset(ones16, 1.0)
            # touch Exp once so the activation table is resident before
            # the first real exp on the phase-2 critical path
            warm = constp.tile([P, 1], F32)
            nc.scalar.activation(
                warm, ones16, mybir.ActivationFunctionType.Exp,
                bias=0.0, scale=1.0,
            )

            xt = persist.tile([P, HT, N], BF16)  # X^T [h, i] (K-side operand)
            gt = persist.tile([P, HT, N], BF16)  # G^T [h2, i] (Q-side)
            vt = persist.tile([P, NT, E], BF16)  # V   [j, e]

            with (
                tc.tile_pool(name="ph1", bufs=1) as ph1,
                tc.tile_pool(name="stage", bufs=3) as stage,
                tc.tile_pool(name="psA", bufs=5, space="PSUM") as psA,
                tc.tile_pool(name="psT", bufs=3, space="PSUM") as psT,
            ):
                wvb = ph1.tile([P, HT, E], BF16, tag="wv")  # Wv natural [h, e]
                wqT = ph1.tile([P, ET, H], BF16, tag="wqT")  # Wq^T [e, h1]
                wkT = ph1.tile([P, ET, H], BF16, tag="wkT")  # Wk^T [e, h2]
                ab = ph1.tile([P, HT, H], BF16, tag="A")  # A [h1, h2]

                def emit_wv(ho):
                    ws = stage.tile([P, E], F32, tag="wst")
                    nc.sync.dma_start(ws, wv[ho * P:(ho + 1) * P, :])
                    nc.vector.tensor_copy(wvb[:, ho, :], ws)

                def emit_x_tile(it, split_first=False):
                    for hh in range(2):
                        if split_first and hh == 0:
                            # 4 small DMAs so the very first cast/transpose
                            # starts ~1.3us earlier at kernel launch
                            xb = stage.tile([P, H // 2], BF16, tag="xbt")
                            for hi in range(HT // 2):
                                xs = stage.tile([P, P], F32, tag="xs0")
                                nc.sync.dma_start(
                                    xs, x[it * P:(it + 1) * P,
                                          hi * P:(hi + 1) * P])
                                nc.scalar.copy(
                                    xb[:, hi * P:(hi + 1) * P], xs)
                        else:
                            xs = stage.tile([P, H // 2], F32, tag="xst")
                            nc.sync.dma_start(
                                xs, x[it * P:(it + 1) * P,
                                      hh * (H // 2):(hh + 1) * (H // 2)])
                            xb = stage.tile([P, H // 2], BF16, tag="xbt")
                            nc.scalar.copy(xb, xs)
                        tp4 = psT.tile([P, 4, P], BF16, tag="tp")
                        for hi in range(HT // 2):
                            nc.tensor.transpose(
                                tp4[:, hi, :], xb[:, hi * P:(hi + 1) * P],
                                ident16)
                        nc.vector.tensor_copy(
                            xt[:, 4 * hh:4 * (hh + 1), it * P:(it + 1) * P],
                            tp4)

                def emit_w_chunk(wsrc, wdstT, ho):
                    """One [128, 1024] row chunk of wq/wk: DMA, cast, 8 PE
                    transposes (batched 4 per PSUM bank + single copy)."""
                    ws = stage.tile([P, E], F32, tag="wst")
                    nc.sync.dma_start(ws, wsrc[ho * P:(ho + 1) * P, :])
                    wb = stage.tile([P, E], BF16, tag="wbt")
                    nc.scalar.copy(wb, ws)
                    for eh in range(2):
                        tp4 = psT.tile([P, 4, P], BF16, tag="tp")
                        for ei in range(4):
                            nc.tensor.transpose(
                                tp4[:, ei, :],
                                wb[:, (4 * eh + ei) * P:(4 * eh + ei + 1) * P],
                                ident16)
                        nc.vector.tensor_copy(
                            wdstT[:, 4 * eh:4 * (eh + 1),
                                  ho * P:(ho + 1) * P], tp4)

                def emit_v_proj(jt):
                    for es in range(E // 512):
                        mm = psA.tile([P, 512], F32, tag="mm")
                        for ho in range(HT):
                            nc.tensor.matmul(
                                mm,
                                lhsT=xt[:, ho, jt * P:(jt + 1) * P],
                                rhs=wvb[:, ho, es * 512:(es + 1) * 512],
                                start=(ho == 0), stop=(ho == HT - 1),
                            )
                        nc.vector.tensor_copy(
                            vt[:, jt, es * 512:(es + 1) * 512], mm)

                # DMA order: x0 first (transposes start immediately), then
                # all wv (V(0)'s matmuls dribble in per arriving chunk - the
                # per-operand deps handle it), then the remaining X tiles
                # with V-projection lagging one tile, wq/wk streaming into
                # the back half.
                wlist = [(wq, wqT, ho) for ho in range(HT)] + \
                        [(wk, wkT, ho) for ho in range(HT)]
                emit_x_tile(0)
                emit_x_tile(1)
                for ho in range(HT):
                    emit_wv(ho)
                emit_v_proj(0)
                emit_v_proj(1)
                for it in range(2, NT):
                    emit_x_tile(it)
                    if it >= 3:
                        emit_v_proj(it - 1)
                    if it >= 8:
                        for c in (2 * (it - 8), 2 * (it - 8) + 1):
                            emit_w_chunk(*wlist[c])
                emit_v_proj(NT - 1)

                # ---- A = Wq Wk^T : A[h1, h2] = sum_e WqT[e,h1] WkT[e,h2]
                for h1t in range(HT):
                    for h2s in range(H // 512):
                        mm = psA.tile([P, 512], F32, tag="mm")
                        for et in range(ET):
                            nc.tensor.matmul(
                                mm,
                                lhsT=wqT[:, et, h1t * P:(h1t + 1) * P],
                                rhs=wkT[:, et, h2s * 512:(h2s + 1) * 512],
                                start=(et == 0), stop=(et == ET - 1),
                            )
                        nc.vector.tensor_copy(
                            ab[:, h1t, h2s * 512:(h2s + 1) * 512], mm)

                # ---- GT[h2, i] = sum_h1 A[h1, h2] XT[h1, i] ----
                for ns in range(N // 512):
                    for h2t in range(HT):
                        mm = psA.tile([P, 512], F32, tag="mm")
                        for h1t in range(HT):
                            nc.tensor.matmul(
                                mm,
                                lhsT=ab[:, h1t, h2t * P:(h2t + 1) * P],
                                rhs=xt[:, h1t, ns * 512:(ns + 1) * 512],
                                start=(h1t == 0), stop=(h1t == HT - 1),
                            )
                        nc.vector.tensor_copy(
                            gt[:, h2t, ns * 512:(ns + 1) * 512], mm)

            # ---- attention: row-block pairs (it0, it0+1), ascending ----
            with (
                tc.tile_pool(name="work", bufs=4) as work,
                tc.tile_pool(name="obuf", bufs=4) as obuf,
                tc.tile_pool(name="accp", bufs=2) as accp,
                tc.tile_pool(name="psS", bufs=2, space="PSUM") as psS,
                tc.tile_pool(name="psO", bufs=4, space="PSUM") as psO,
                tc.tile_pool(name="psR", bufs=2, space="PSUM") as psR,
            ):
                # row-block pairs, except the last two blocks run as
                # singles: block 14's output drain then overlaps block 15's
                # compute, shortening the end-of-kernel tail.
                groups = [[2 * p, 2 * p + 1] for p in range(NT // 2 - 1)]
                groups += [[NT - 2], [NT - 1]]
                for gi, grp in enumerate(groups):
                    it0 = grp[0]
                    i0 = it0 * P
                    gw = len(grp) * P

                    def emit_st(jt, it0=it0, i0=i0, gw=gw, grp=grp):
                        """ST[j, i0:i0+w] for unit (group, jt). First unit
                        covers only the low block (high block fully masked).
                        Mask lands as an extra matmul inside the group."""
                        w = P if jt == it0 else gw
                        diag = jt in grp
                        sp = psS.tile([P, 2 * P], F32, tag="s")
                        for et in range(ET):
                            nc.tensor.matmul(
                                sp[:, :w],
                                lhsT=xt[:, et, jt * P:(jt + 1) * P],
                                rhs=gt[:, et, i0:i0 + w],
                                start=(et == 0),
                                stop=(et == ET - 1) and not diag,
                            )
                        if diag:
                            off = (jt - it0) * P
                            nc.tensor.matmul(
                                sp[:, off:off + P], lhsT=ident16, rhs=maskR,
                                start=False, stop=True,
                            )
                        pb = work.tile([P, 2 * P], BF16, tag="p")
                        nc.scalar.activation(
                            pb[:, :w], sp[:, :w],
                            mybir.ActivationFunctionType.Exp,
                            bias=0.0, scale=SCALE,
                        )
                        return pb

                    obanks = []
                    for itx in grp:
                        obanks.append((
                            itx,
                            psO.tile([P, 512], F32, tag="o", name=f"o0_{itx}"),
                            psO.tile([P, 512], F32, tag="o", name=f"o1_{itx}"),
                            psR.tile([P, 1], F32, tag="rs",
                                     name=f"rs_{itx}")))

                    pb_prev = emit_st(it0)
                    for jt in range(it0, NT):
                        pb = pb_prev
                        if jt + 1 < NT:
                            pb_prev = emit_st(jt + 1)
                        last = jt == NT - 1
                        for bi, (itx, o0, o1, rsx) in enumerate(obanks):
                            if jt < itx:
                                continue
                            first = jt == itx
                            sl = pb[:, bi * P:(bi + 1) * P]
                            nc.tensor.matmul(
                                o0, lhsT=sl, rhs=vt[:, jt, 0:512],
                                start=first, stop=last)
                            nc.tensor.matmul(
                                o1, lhsT=sl, rhs=vt[:, jt, 512:1024],
                                start=first, stop=last)
                            nc.tensor.matmul(
                                rsx, lhsT=sl, rhs=ones16,
                                start=first, stop=last)

                    # scale + drain on DVE (Act stays clear for the next
                    # group's exp chain); low block first - its PSUM banks
                    # are the ones the next group needs soonest. The final
                    # block splits its two scalings across DVE and Act.
                    final = gi == len(groups) - 1
                    for (itx, o0, o1, rsx) in obanks:
                        ri = accp.tile([P, 1], F32, tag="ri")
                        nc.vector.reciprocal(ri, rsx)
                        for es, op in enumerate((o0, o1)):
                            ob = obuf.tile([P, 512], F32, tag="ob")
                            if final and es == 1:
                                nc.scalar.mul(ob, op, ri)
                            else:
                                nc.vector.tensor_scalar_mul(ob, op, ri)
                            nc.sync.dma_start(
                                out[itx * P:(itx + 1) * P,
                                    es * 512:(es + 1) * 512], ob)

    nc.finalize()
    return nc


_NC = None


def _get_nc():
    global _NC
    if _NC is None:
        _NC = build_graph()
    return _NC


def _run(inputs, trace=False, **kwargs):
    x = np.ascontiguousarray(np.asarray(inputs["input"], dtype=np.float32))
    k = np.ascontiguousarray(np.asarray(inputs["k"], dtype=np.float32))
    q = np.ascontiguousarray(np.asarray(inputs["q"], dtype=np.float32))
    v = np.ascontiguousarray(np.asarray(inputs["v"], dtype=np.float32))
    assert x.shape == (B, N, H)
    nc = _get_nc()
    in_maps = [
        {"input": x[b], "k": k, "q": q, "v": v} for b in range(B)
    ]
    res = bass_utils.run_bass_kernel_spmd(
        nc, in_maps, core_ids=list(range(B)), trace=trace, **kwargs)
    outs = np.stack([np.asarray(r["out"]) for r in res.results], axis=0)
    return outs.astype(np.float32), res


def kernel(**inputs):
    outs, _ = _run(inputs, trace=False)
    return outs


# revision 6
# speedup vs baseline: 1.0123x; 1.0111x over previous
"""Distributed Trainium2 kernel for nn_AttentionHead (B=8, N=2048, H=E=1024).

Single attention head, causal mask keeping j >= i, softmax over j, per batch:

    K = X Wk; Q = X Wq; V = X Wv
    S = Q K^T / sqrt(E);  S[i, j] = -inf for i > j
    O = softmax_j(S) V

Sharding: pure data parallel - batch b (8) maps 1:1 onto the 8 NeuronCores.
Weights replicated; no collectives.

Per-core algorithm (matmuls bf16, fp32 PSUM):
  Phase 1 (streaming):
    - DMA X tiles with wv chunks interleaved, then wq, wk (one SP queue).
    - X/Wq/Wk cast bf16 (ScalarE), PE-transposed; 4 transposes share one
      PSUM bank and drain with a single DVE copy (DVE instruction count
      is the phase-1 drag otherwise).
    - V = X Wv per row tile, interleaved with the X stream; Wq/Wk chunks
      interleaved into the V tail so their DMA/cast/transpose overlaps PE.
    - A = Wq Wk^T, then GT[h2,i] = sum_h1 A[h1,h2] XT[h1,i]:
      S = Q K^T = X (Wq Wk^T) X^T needs only this one projection.
  Phase 2 (attention, row-block PAIRS, ascending):
    - Scores TRANSPOSED per unit (pair p, j-tile jt): ST[j, i0:i0+256]
      covering both blocks of the pair; 256-wide rhs streams amortize
      weight loads and give exp 2x the hiding window.
    - Causal mask applied INSIDE the score matmul group: one extra
      matmul with lhsT=identity, rhs=precomputed NEG upper-strict tile
      adds -1e30 to masked entries - no cross-engine mask hop.
    - exp on ScalarE writes PT[j,i] bf16 straight to SBUF: PT is the
      ready-made stationary operand for PV (no P transposes).
    - Row sums via 1-column ones-matmuls sharing PV's loaded weights.
    - PSUM banks: 2 x ST + 4 x O (pair) + 2 x rowsum = 8.
    - O rows scaled by 1/rowsum (ScalarE), DMA out per 512 columns.
"""

import numpy as np

try:
    import concourse.bass as bass
except ImportError:  # fresh grading dir: concourse comes from the site repo
    import sys

    for p in ("/opt/trn_rl_repo", "/root/.axon_site/_ro/trn_rl_repo"):
        if p not in sys.path:
            sys.path.append(p)
    import concourse.bass as bass

import concourse.mybir as mybir
import concourse.tile as tile
from concourse import bacc, bass_utils
from concourse.masks import make_identity

B, N, H, E = 8, 2048, 1024, 1024
P = 128
HT = H // P  # 8 h-tiles
ET = E // P  # 8 e-tiles
NT = N // P  # 16 row tiles
F32 = mybir.dt.float32
BF16 = mybir.dt.bfloat16
SCALE = 1.0 / float(np.sqrt(E))
NEG = -1.0e30


def build_graph():
    nc = bacc.Bacc("TRN2", target_bir_lowering=False, debug=False,
                   enable_asserts=False)
    x = nc.dram_tensor("input", [N, H], F32, kind="ExternalInput").ap()
    wk = nc.dram_tensor("k", [H, E], F32, kind="ExternalInput").ap()
    wq = nc.dram_tensor("q", [H, E], F32, kind="ExternalInput").ap()
    wv = nc.dram_tensor("v", [H, E], F32, kind="ExternalInput").ap()
    out = nc.dram_tensor("out", [N, E], F32, kind="ExternalOutput").ap()

    with tile.TileContext(nc) as tc:
        with (
            tc.tile_pool(name="const", bufs=1) as constp,
            tc.tile_pool(name="persist", bufs=1) as persist,
        ):
            ident16 = constp.tile([P, P], BF16)
            make_identity(nc, ident16)
            # maskR[p, i] = NEG where p < i else 0. Matmul with
            # lhsT=ident16 reproduces it into ST[j, i]: adds NEG to the
            # strictly-masked (j < i) entries of a diagonal unit.
            maskR = constp.tile([P, P], BF16)
            nc.gpsimd.memset(maskR, 0.0)
            # keep 0 where iota = p - i >= 0, fill NEG where p < i
            # (is_ge is the only comparison the neuronx backend implements
            # for affine_select)
            nc.gpsimd.affine_select(
                out=maskR, in_=maskR, compare_op=mybir.AluOpType.is_ge,
                fill=NEG, base=0, pattern=[[-1, P]], channel_multiplier=1,
            )
            ones16 = constp.tile([P, 1], BF16)
            nc.gpsimd.memset(ones16, 1.0)
            # touch Exp once so the activation table is resident before
            # the first real exp on the phase-2 critical path
            warm = constp.tile([P, 1], F32)
            nc.scalar.activation(
                warm, ones16, mybir.ActivationFunctionType.Exp,
                bias=0.0, scale=1.0,
            )

            xt = persist.tile([P, HT, N], BF16)  # X^T [h, i] (K-side operand)
            gt = persist.tile([P, HT, N], BF16)  # G^T [h2, i] (Q-side)
            vt = persist.tile([P, NT, E], BF16)  # V   [j, e]

            with (
                tc.tile_pool(name="ph1", bufs=1) as ph1,
                tc.tile_pool(name="stage", bufs=3) as stage,
                tc.tile_pool(name="psA", bufs=5, space="PSUM") as psA,
                tc.tile_pool(name="psT", bufs=3, space="PSUM") as psT,
            ):
                wvb = ph1.tile([P, HT, E], BF16, tag="wv")  # Wv natural [h, e]
                wqT = ph1.tile([P, ET, H], BF16, tag="wqT")  # Wq^T [e, h1]
                wkT = ph1.tile([P, ET, H], BF16, tag="wkT")  # Wk^T [e, h2]
                ab = ph1.tile([P, HT, H], BF16, tag="A")  # A [h1, h2]

                def emit_wv_half(ho, es):
                    ws = stage.tile([P, 512], F32, tag="wvh")
                    nc.sync.dma_start(
                        ws, wv[ho * P:(ho + 1) * P, es * 512:(es + 1) * 512])
                    nc.vector.tensor_copy(
                        wvb[:, ho, es * 512:(es + 1) * 512], ws)

                def emit_x_tile(it, split_first=False):
                    for hh in range(2):
                        if split_first and hh == 0:
                            # 4 small DMAs so the very first cast/transpose
                            # starts ~1.3us earlier at kernel launch
                            xb = stage.tile([P, H // 2], BF16, tag="xbt")
                            for hi in range(HT // 2):
                                xs = stage.tile([P, P], F32, tag="xs0")
                                nc.sync.dma_start(
                                    xs, x[it * P:(it + 1) * P,
                                          hi * P:(hi + 1) * P])
                                nc.scalar.copy(
                                    xb[:, hi * P:(hi + 1) * P], xs)
                        else:
                            xs = stage.tile([P, H // 2], F32, tag="xst")
                            nc.sync.dma_start(
                                xs, x[it * P:(it + 1) * P,
                                      hh * (H // 2):(hh + 1) * (H // 2)])
                            xb = stage.tile([P, H // 2], BF16, tag="xbt")
                            nc.scalar.copy(xb, xs)
                        tp4 = psT.tile([P, 4, P], BF16, tag="tp")
                        for hi in range(HT // 2):
                            nc.tensor.transpose(
                                tp4[:, hi, :], xb[:, hi * P:(hi + 1) * P],
                                ident16)
                        nc.vector.tensor_copy(
                            xt[:, 4 * hh:4 * (hh + 1), it * P:(it + 1) * P],
                            tp4)

                def emit_w_chunk(wsrc, wdstT, ho):
                    """One [128, 1024] row chunk of wq/wk: DMA, cast, 8 PE
                    transposes (batched 4 per PSUM bank + single copy)."""
                    ws = stage.tile([P, E], F32, tag="wst")
                    nc.sync.dma_start(ws, wsrc[ho * P:(ho + 1) * P, :])
                    wb = stage.tile([P, E], BF16, tag="wbt")
                    nc.scalar.copy(wb, ws)
                    for eh in range(2):
                        tp4 = psT.tile([P, 4, P], BF16, tag="tp")
                        for ei in range(4):
                            nc.tensor.transpose(
                                tp4[:, ei, :],
                                wb[:, (4 * eh + ei) * P:(4 * eh + ei + 1) * P],
                                ident16)
                        nc.vector.tensor_copy(
                            wdstT[:, 4 * eh:4 * (eh + 1),
                                  ho * P:(ho + 1) * P], tp4)

                def emit_v_es(jt, es):
                    mm = psA.tile([P, 512], F32, tag="mm")
                    for ho in range(HT):
                        nc.tensor.matmul(
                            mm,
                            lhsT=xt[:, ho, jt * P:(jt + 1) * P],
                            rhs=wvb[:, ho, es * 512:(es + 1) * 512],
                            start=(ho == 0), stop=(ho == HT - 1),
                        )
                    nc.vector.tensor_copy(
                        vt[:, jt, es * 512:(es + 1) * 512], mm)

                def emit_v_proj(jt):
                    for es in range(E // 512):
                        emit_v_es(jt, es)

                # DMA order: x0 first (transposes start immediately), then
                # all wv (V(0)'s matmuls dribble in per arriving chunk - the
                # per-operand deps handle it), then the remaining X tiles
                # with V-projection lagging one tile, wq/wk streaming into
                # the back half.
                wlist = [(wq, wqT, ho) for ho in range(HT)] + \
                        [(wk, wkT, ho) for ho in range(HT)]
                # wv streams in 512-col halves: the es0 V-projection groups
                # need only the first half (2.1MB not 4.2MB), halving the
                # data the PE start-up is gated on; es1 slots in later.
                emit_x_tile(0)
                emit_x_tile(1)
                for ho in range(HT):
                    emit_wv_half(ho, 0)
                emit_v_es(0, 0)
                emit_v_es(1, 0)
                emit_x_tile(2)
                for ho in range(HT // 2):
                    emit_wv_half(ho, 1)
                emit_x_tile(3)
                for ho in range(HT // 2, HT):
                    emit_wv_half(ho, 1)
                emit_v_es(0, 1)
                emit_v_es(1, 1)
                for it in range(4, NT):
                    emit_x_tile(it)
                    emit_v_proj(it - 2)
                    if it >= 8:
                        for c in (2 * (it - 8), 2 * (it - 8) + 1):
                            emit_w_chunk(*wlist[c])
                emit_v_proj(NT - 2)
                emit_v_proj(NT - 1)

                # ---- A = Wq Wk^T : A[h1, h2] = sum_e WqT[e,h1] WkT[e,h2]
                for h1t in range(HT):
                    for h2s in range(H // 512):
                        mm = psA.tile([P, 512], F32, tag="mm")
                        for et in range(ET):
                            nc.tensor.matmul(
                                mm,
                                lhsT=wqT[:, et, h1t * P:(h1t + 1) * P],
                                rhs=wkT[:, et, h2s * 512:(h2s + 1) * 512],
                                start=(et == 0), stop=(et == ET - 1),
                            )
                        nc.vector.tensor_copy(
                            ab[:, h1t, h2s * 512:(h2s + 1) * 512], mm)

                # ---- GT[h2, i] = sum_h1 A[h1, h2] XT[h1, i] ----
                pb0 = persist.tile([P, 2 * P], BF16, name="pb0")
                for ns in range(N // 512):
                    if ns == N // 512 - 1:
                        # prefetch attention unit (block 0, jt 0): only
                        # needs gt columns 0:128 (done since ns=0), and its
                        # exp hides under the whole last GT chunk - kills
                        # the phase-transition pipeline-fill stall.
                        sp0 = psA.tile([P, 512], F32, tag="mm", name="sp0")
                        for et in range(ET):
                            nc.tensor.matmul(
                                sp0[:, :P],
                                lhsT=xt[:, et, 0:P], rhs=gt[:, et, 0:P],
                                start=(et == 0), stop=False,
                            )
                        nc.tensor.matmul(
                            sp0[:, 0:P], lhsT=ident16, rhs=maskR,
                            start=False, stop=True,
                        )
                        nc.scalar.activation(
                            pb0[:, :P], sp0[:, :P],
                            mybir.ActivationFunctionType.Exp,
                            bias=0.0, scale=SCALE,
                        )
                    for h2t in range(HT):
                        mm = psA.tile([P, 512], F32, tag="mm")
                        for h1t in range(HT):
                            nc.tensor.matmul(
                                mm,
                                lhsT=ab[:, h1t, h2t * P:(h2t + 1) * P],
                                rhs=xt[:, h1t, ns * 512:(ns + 1) * 512],
                                start=(h1t == 0), stop=(h1t == HT - 1),
                            )
                        nc.vector.tensor_copy(
                            gt[:, h2t, ns * 512:(ns + 1) * 512], mm)

            # ---- attention: row-block pairs (it0, it0+1), ascending ----
            with (
                tc.tile_pool(name="work", bufs=4) as work,
                tc.tile_pool(name="obuf", bufs=4) as obuf,
                tc.tile_pool(name="accp", bufs=2) as accp,
                tc.tile_pool(name="psS", bufs=2, space="PSUM") as psS,
                tc.tile_pool(name="psO", bufs=4, space="PSUM") as psO,
                tc.tile_pool(name="psR", bufs=2, space="PSUM") as psR,
            ):
                # row-block pairs, except the last two blocks run as
                # singles: block 14's output drain then overlaps block 15's
                # compute, shortening the end-of-kernel tail.
                groups = [[2 * p, 2 * p + 1] for p in range(NT // 2 - 1)]
                groups += [[NT - 2], [NT - 1]]
                for gi, grp in enumerate(groups):
                    it0 = grp[0]
                    i0 = it0 * P
                    gw = len(grp) * P

                    def emit_st(jt, it0=it0, i0=i0, gw=gw, grp=grp):
                        """ST[j, i0:i0+w] for unit (group, jt). First unit
                        covers only the low block (high block fully masked).
                        Mask lands as an extra matmul inside the group."""
                        w = P if jt == it0 else gw
                        diag = jt in grp
                        sp = psS.tile([P, 2 * P], F32, tag="s")
                        for et in range(ET):
                            nc.tensor.matmul(
                                sp[:, :w],
                                lhsT=xt[:, et, jt * P:(jt + 1) * P],
                                rhs=gt[:, et, i0:i0 + w],
                                start=(et == 0),
                                stop=(et == ET - 1) and not diag,
                            )
                        if diag:
                            off = (jt - it0) * P
                            nc.tensor.matmul(
                                sp[:, off:off + P], lhsT=ident16, rhs=maskR,
                                start=False, stop=True,
                            )
                        pb = work.tile([P, 2 * P], BF16, tag="p")
                        nc.scalar.activation(
                            pb[:, :w], sp[:, :w],
                            mybir.ActivationFunctionType.Exp,
                            bias=0.0, scale=SCALE,
                        )
                        return pb

                    obanks = []
                    for itx in grp:
                        obanks.append((
                            itx,
                            psO.tile([P, 512], F32, tag="o", name=f"o0_{itx}"),
                            psO.tile([P, 512], F32, tag="o", name=f"o1_{itx}"),
                            psR.tile([P, 1], F32, tag="rs",
                                     name=f"rs_{itx}")))

                    pb_prev = pb0 if gi == 0 else emit_st(it0)
                    for jt in range(it0, NT):
                        pb = pb_prev
                        if jt + 1 < NT:
                            pb_prev = emit_st(jt + 1)
                        last = jt == NT - 1
                        for bi, (itx, o0, o1, rsx) in enumerate(obanks):
                            if jt < itx:
                                continue
                            first = jt == itx
                            sl = pb[:, bi * P:(bi + 1) * P]
                            nc.tensor.matmul(
                                o0, lhsT=sl, rhs=vt[:, jt, 0:512],
                                start=first, stop=last)
                            nc.tensor.matmul(
                                o1, lhsT=sl, rhs=vt[:, jt, 512:1024],
                                start=first, stop=last)
                            nc.tensor.matmul(
                                rsx, lhsT=sl, rhs=ones16,
                                start=first, stop=last)

                    # scale + drain on DVE (Act stays clear for the next
                    # group's exp chain); low block first - its PSUM banks
                    # are the ones the next group needs soonest. The final
                    # block splits its two scalings across DVE and Act.
                    final = gi == len(groups) - 1
                    for (itx, o0, o1, rsx) in obanks:
                        ri = accp.tile([P, 1], F32, tag="ri")
                        nc.vector.reciprocal(ri, rsx)
                        for es, op in enumerate((o0, o1)):
                            ob = obuf.tile([P, 512], F32, tag="ob")
                            if final and es == 1:
                                nc.scalar.mul(ob, op, ri)
                            else:
                                nc.vector.tensor_scalar_mul(ob, op, ri)
                            nc.sync.dma_start(
                                out[itx * P:(itx + 1) * P,
                                    es * 512:(es + 1) * 512], ob)

    nc.finalize()
    return nc


_NC = None


def _get_nc():
    global _NC
    if _NC is None:
        _NC = build_graph()
    return _NC


def _run(inputs, trace=False, **kwargs):
    x = np.ascontiguousarray(np.asarray(inputs["input"], dtype=np.float32))
    k = np.ascontiguousarray(np.asarray(inputs["k"], dtype=np.float32))
    q = np.ascontiguousarray(np.asarray(inputs["q"], dtype=np.float32))
    v = np.ascontiguousarray(np.asarray(inputs["v"], dtype=np.float32))
    assert x.shape == (B, N, H)
    nc = _get_nc()
    in_maps = [
        {"input": x[b], "k": k, "q": q, "v": v} for b in range(B)
    ]
    res = bass_utils.run_bass_kernel_spmd(
        nc, in_maps, core_ids=list(range(B)), trace=trace, **kwargs)
    outs = np.stack([np.asarray(r["out"]) for r in res.results], axis=0)
    return outs.astype(np.float32), res


def kernel(**inputs):
    outs, _ = _run(inputs, trace=False)
    return outs


# revision 7
# speedup vs baseline: 1.0193x; 1.0070x over previous
"""Distributed Trainium2 kernel for nn_AttentionHead (B=8, N=2048, H=E=1024).

Single attention head, causal mask keeping j >= i, softmax over j, per batch:

    K = X Wk; Q = X Wq; V = X Wv
    S = Q K^T / sqrt(E);  S[i, j] = -inf for i > j
    O = softmax_j(S) V

Sharding: pure data parallel - batch b (8) maps 1:1 onto the 8 NeuronCores.
Weights replicated; no collectives.

Per-core algorithm (matmuls bf16, fp32 PSUM):
  Phase 1 (streaming):
    - DMA X tiles with wv chunks interleaved, then wq, wk (one SP queue).
    - X/Wq/Wk cast bf16 (ScalarE), PE-transposed; 4 transposes share one
      PSUM bank and drain with a single DVE copy (DVE instruction count
      is the phase-1 drag otherwise).
    - V = X Wv per row tile, interleaved with the X stream; Wq/Wk chunks
      interleaved into the V tail so their DMA/cast/transpose overlaps PE.
    - A = Wq Wk^T, then GT[h2,i] = sum_h1 A[h1,h2] XT[h1,i]:
      S = Q K^T = X (Wq Wk^T) X^T needs only this one projection.
  Phase 2 (attention, row-block PAIRS, ascending):
    - Scores TRANSPOSED per unit (pair p, j-tile jt): ST[j, i0:i0+256]
      covering both blocks of the pair; 256-wide rhs streams amortize
      weight loads and give exp 2x the hiding window.
    - Causal mask applied INSIDE the score matmul group: one extra
      matmul with lhsT=identity, rhs=precomputed NEG upper-strict tile
      adds -1e30 to masked entries - no cross-engine mask hop.
    - exp on ScalarE writes PT[j,i] bf16 straight to SBUF: PT is the
      ready-made stationary operand for PV (no P transposes).
    - Row sums via 1-column ones-matmuls sharing PV's loaded weights.
    - PSUM banks: 2 x ST + 4 x O (pair) + 2 x rowsum = 8.
    - O rows scaled by 1/rowsum (ScalarE), DMA out per 512 columns.
"""

import numpy as np

try:
    import concourse.bass as bass
except ImportError:  # fresh grading dir: concourse comes from the site repo
    import sys

    for p in ("/opt/trn_rl_repo", "/root/.axon_site/_ro/trn_rl_repo"):
        if p not in sys.path:
            sys.path.append(p)
    import concourse.bass as bass

import concourse.mybir as mybir
import concourse.tile as tile
from concourse import bacc, bass_utils
from concourse.masks import make_identity

B, N, H, E = 8, 2048, 1024, 1024
P = 128
HT = H // P  # 8 h-tiles
ET = E // P  # 8 e-tiles
NT = N // P  # 16 row tiles
F32 = mybir.dt.float32
BF16 = mybir.dt.bfloat16
SCALE = 1.0 / float(np.sqrt(E))
NEG = -1.0e30


def build_graph():
    nc = bacc.Bacc("TRN2", target_bir_lowering=False, debug=False,
                   enable_asserts=False)
    x = nc.dram_tensor("input", [N, H], F32, kind="ExternalInput").ap()
    wk = nc.dram_tensor("k", [H, E], F32, kind="ExternalInput").ap()
    wq = nc.dram_tensor("q", [H, E], F32, kind="ExternalInput").ap()
    wv = nc.dram_tensor("v", [H, E], F32, kind="ExternalInput").ap()
    out = nc.dram_tensor("out", [N, E], F32, kind="ExternalOutput").ap()

    with tile.TileContext(nc) as tc:
        with (
            tc.tile_pool(name="const", bufs=1) as constp,
            tc.tile_pool(name="persist", bufs=1) as persist,
        ):
            ident16 = constp.tile([P, P], BF16)
            make_identity(nc, ident16)
            # maskR[p, i] = NEG where p < i else 0. Matmul with
            # lhsT=ident16 reproduces it into ST[j, i]: adds NEG to the
            # strictly-masked (j < i) entries of a diagonal unit.
            maskR = constp.tile([P, P], BF16)
            nc.gpsimd.memset(maskR, 0.0)
            # keep 0 where iota = p - i >= 0, fill NEG where p < i
            # (is_ge is the only comparison the neuronx backend implements
            # for affine_select)
            nc.gpsimd.affine_select(
                out=maskR, in_=maskR, compare_op=mybir.AluOpType.is_ge,
                fill=NEG, base=0, pattern=[[-1, P]], channel_multiplier=1,
            )
            ones16 = constp.tile([P, 1], BF16)
            nc.gpsimd.memset(ones16, 1.0)
            # touch Exp once so the activation table is resident before
            # the first real exp on the phase-2 critical path
            warm = constp.tile([P, 1], F32)
            nc.scalar.activation(
                warm, ones16, mybir.ActivationFunctionType.Exp,
                bias=0.0, scale=1.0,
            )

            xt = persist.tile([P, HT, N], BF16)  # X^T [h, i] (K-side operand)
            gt = persist.tile([P, HT, N], BF16)  # G^T [h2, i] (Q-side)
            vt = persist.tile([P, NT, E], BF16)  # V   [j, e]

            with (
                tc.tile_pool(name="ph1", bufs=1) as ph1,
                tc.tile_pool(name="stage", bufs=3) as stage,
                tc.tile_pool(name="psA", bufs=5, space="PSUM") as psA,
                tc.tile_pool(name="psT", bufs=3, space="PSUM") as psT,
            ):
                wvb = ph1.tile([P, HT, E], BF16, tag="wv")  # Wv natural [h, e]
                wqT = ph1.tile([P, ET, H], BF16, tag="wqT")  # Wq^T [e, h1]
                wkT = ph1.tile([P, ET, H], BF16, tag="wkT")  # Wk^T [e, h2]
                ab = ph1.tile([P, HT, H], BF16, tag="A")  # A [h1, h2]

                def emit_wv_half(ho, es):
                    ws = stage.tile([P, 512], F32, tag="wvh")
                    nc.sync.dma_start(
                        ws, wv[ho * P:(ho + 1) * P, es * 512:(es + 1) * 512])
                    # copy on the idle GpSimd queue: on the DVE these sit
                    # behind xt-transpose copies that wait on later DMAs
                    # (head-of-line blocking) and starve the V dribble
                    nc.gpsimd.tensor_copy(
                        wvb[:, ho, es * 512:(es + 1) * 512], ws)

                def emit_x_tile(it, split_first=False):
                    for hh in range(2):
                        if split_first and hh == 0:
                            # 4 small DMAs so the very first cast/transpose
                            # starts ~1.3us earlier at kernel launch
                            xb = stage.tile([P, H // 2], BF16, tag="xbt")
                            for hi in range(HT // 2):
                                xs = stage.tile([P, P], F32, tag="xs0")
                                nc.sync.dma_start(
                                    xs, x[it * P:(it + 1) * P,
                                          hi * P:(hi + 1) * P])
                                nc.scalar.copy(
                                    xb[:, hi * P:(hi + 1) * P], xs)
                        else:
                            xs = stage.tile([P, H // 2], F32, tag="xst")
                            nc.sync.dma_start(
                                xs, x[it * P:(it + 1) * P,
                                      hh * (H // 2):(hh + 1) * (H // 2)])
                            xb = stage.tile([P, H // 2], BF16, tag="xbt")
                            nc.scalar.copy(xb, xs)
                        tp4 = psT.tile([P, 4, P], BF16, tag="tp")
                        for hi in range(HT // 2):
                            nc.tensor.transpose(
                                tp4[:, hi, :], xb[:, hi * P:(hi + 1) * P],
                                ident16)
                        nc.vector.tensor_copy(
                            xt[:, 4 * hh:4 * (hh + 1), it * P:(it + 1) * P],
                            tp4)

                def emit_w_chunk(wsrc, wdstT, ho):
                    """One [128, 1024] row chunk of wq/wk: DMA, cast, 8 PE
                    transposes (batched 4 per PSUM bank + single copy)."""
                    ws = stage.tile([P, E], F32, tag="wst")
                    nc.sync.dma_start(ws, wsrc[ho * P:(ho + 1) * P, :])
                    wb = stage.tile([P, E], BF16, tag="wbt")
                    nc.scalar.copy(wb, ws)
                    for eh in range(2):
                        tp4 = psT.tile([P, 4, P], BF16, tag="tp")
                        for ei in range(4):
                            nc.tensor.transpose(
                                tp4[:, ei, :],
                                wb[:, (4 * eh + ei) * P:(4 * eh + ei + 1) * P],
                                ident16)
                        nc.vector.tensor_copy(
                            wdstT[:, 4 * eh:4 * (eh + 1),
                                  ho * P:(ho + 1) * P], tp4)

                def emit_v_es(jt, es):
                    mm = psA.tile([P, 512], F32, tag="mm")
                    for ho in range(HT):
                        nc.tensor.matmul(
                            mm,
                            lhsT=xt[:, ho, jt * P:(jt + 1) * P],
                            rhs=wvb[:, ho, es * 512:(es + 1) * 512],
                            start=(ho == 0), stop=(ho == HT - 1),
                        )
                    nc.vector.tensor_copy(
                        vt[:, jt, es * 512:(es + 1) * 512], mm)

                def emit_v_proj(jt):
                    for es in range(E // 512):
                        emit_v_es(jt, es)

                # DMA order: x0 first (transposes start immediately), then
                # all wv (V(0)'s matmuls dribble in per arriving chunk - the
                # per-operand deps handle it), then the remaining X tiles
                # with V-projection lagging one tile, wq/wk streaming into
                # the back half.
                wlist = [(wq, wqT, ho) for ho in range(HT)] + \
                        [(wk, wkT, ho) for ho in range(HT)]
                # wv streams in 512-col halves: the es0 V-projection groups
                # need only the first half (2.1MB not 4.2MB) of Wv. Four X
                # tiles load first so V(0..3) matmuls dribble through both
                # wv half-windows (per-operand deps let each matmul fire as
                # its chunk lands), keeping the PE fed during the DMA-bound
                # start.
                for it in range(4):
                    emit_x_tile(it)
                for ho in range(HT):
                    emit_wv_half(ho, 0)
                for jt in range(4):
                    emit_v_es(jt, 0)
                for ho in range(HT):
                    emit_wv_half(ho, 1)
                for jt in range(4):
                    emit_v_es(jt, 1)
                for it in range(4, NT):
                    emit_x_tile(it)
                    if it >= 6:
                        emit_v_proj(it - 2)
                    if it >= 8:
                        for c in (2 * (it - 8), 2 * (it - 8) + 1):
                            emit_w_chunk(*wlist[c])
                emit_v_proj(NT - 2)
                emit_v_proj(NT - 1)

                # ---- A = Wq Wk^T : A[h1, h2] = sum_e WqT[e,h1] WkT[e,h2]
                for h1t in range(HT):
                    for h2s in range(H // 512):
                        mm = psA.tile([P, 512], F32, tag="mm")
                        for et in range(ET):
                            nc.tensor.matmul(
                                mm,
                                lhsT=wqT[:, et, h1t * P:(h1t + 1) * P],
                                rhs=wkT[:, et, h2s * 512:(h2s + 1) * 512],
                                start=(et == 0), stop=(et == ET - 1),
                            )
                        nc.vector.tensor_copy(
                            ab[:, h1t, h2s * 512:(h2s + 1) * 512], mm)

                # ---- GT[h2, i] = sum_h1 A[h1, h2] XT[h1, i] ----
                pb0 = persist.tile([P, 2 * P], BF16, name="pb0")
                for ns in range(N // 512):
                    if ns == N // 512 - 1:
                        # prefetch attention unit (block 0, jt 0): only
                        # needs gt columns 0:128 (done since ns=0), and its
                        # exp hides under the whole last GT chunk - kills
                        # the phase-transition pipeline-fill stall.
                        sp0 = psA.tile([P, 512], F32, tag="mm", name="sp0")
                        for et in range(ET):
                            nc.tensor.matmul(
                                sp0[:, :P],
                                lhsT=xt[:, et, 0:P], rhs=gt[:, et, 0:P],
                                start=(et == 0), stop=False,
                            )
                        nc.tensor.matmul(
                            sp0[:, 0:P], lhsT=ident16, rhs=maskR,
                            start=False, stop=True,
                        )
                        nc.scalar.activation(
                            pb0[:, :P], sp0[:, :P],
                            mybir.ActivationFunctionType.Exp,
                            bias=0.0, scale=SCALE,
                        )
                    for h2t in range(HT):
                        mm = psA.tile([P, 512], F32, tag="mm")
                        for h1t in range(HT):
                            nc.tensor.matmul(
                                mm,
                                lhsT=ab[:, h1t, h2t * P:(h2t + 1) * P],
                                rhs=xt[:, h1t, ns * 512:(ns + 1) * 512],
                                start=(h1t == 0), stop=(h1t == HT - 1),
                            )
                        nc.vector.tensor_copy(
                            gt[:, h2t, ns * 512:(ns + 1) * 512], mm)

            # ---- attention: row-block pairs (it0, it0+1), ascending ----
            with (
                tc.tile_pool(name="work", bufs=4) as work,
                tc.tile_pool(name="obuf", bufs=4) as obuf,
                tc.tile_pool(name="accp", bufs=2) as accp,
                tc.tile_pool(name="psS", bufs=2, space="PSUM") as psS,
                tc.tile_pool(name="psO", bufs=4, space="PSUM") as psO,
                tc.tile_pool(name="psR", bufs=2, space="PSUM") as psR,
            ):
                # row-block pairs, except the last two blocks run as
                # singles: block 14's output drain then overlaps block 15's
                # compute, shortening the end-of-kernel tail.
                groups = [[2 * p, 2 * p + 1] for p in range(NT // 2 - 1)]
                groups += [[NT - 2], [NT - 1]]
                for gi, grp in enumerate(groups):
                    it0 = grp[0]
                    i0 = it0 * P
                    gw = len(grp) * P

                    def emit_st(jt, it0=it0, i0=i0, gw=gw, grp=grp):
                        """ST[j, i0:i0+w] for unit (group, jt). First unit
                        covers only the low block (high block fully masked).
                        Mask lands as an extra matmul inside the group."""
                        w = P if jt == it0 else gw
                        diag = jt in grp
                        sp = psS.tile([P, 2 * P], F32, tag="s")
                        for et in range(ET):
                            nc.tensor.matmul(
                                sp[:, :w],
                                lhsT=xt[:, et, jt * P:(jt + 1) * P],
                                rhs=gt[:, et, i0:i0 + w],
                                start=(et == 0),
                                stop=(et == ET - 1) and not diag,
                            )
                        if diag:
                            off = (jt - it0) * P
                            nc.tensor.matmul(
                                sp[:, off:off + P], lhsT=ident16, rhs=maskR,
                                start=False, stop=True,
                            )
                        pb = work.tile([P, 2 * P], BF16, tag="p")
                        nc.scalar.activation(
                            pb[:, :w], sp[:, :w],
                            mybir.ActivationFunctionType.Exp,
                            bias=0.0, scale=SCALE,
                        )
                        return pb

                    obanks = []
                    for itx in grp:
                        obanks.append((
                            itx,
                            psO.tile([P, 512], F32, tag="o", name=f"o0_{itx}"),
                            psO.tile([P, 512], F32, tag="o", name=f"o1_{itx}"),
                            psR.tile([P, 1], F32, tag="rs",
                                     name=f"rs_{itx}")))

                    pb_prev = pb0 if gi == 0 else emit_st(it0)
                    for jt in range(it0, NT):
                        pb = pb_prev
                        if jt + 1 < NT:
                            pb_prev = emit_st(jt + 1)
                        last = jt == NT - 1
                        for bi, (itx, o0, o1, rsx) in enumerate(obanks):
                            if jt < itx:
                                continue
                            first = jt == itx
                            sl = pb[:, bi * P:(bi + 1) * P]
                            nc.tensor.matmul(
                                o0, lhsT=sl, rhs=vt[:, jt, 0:512],
                                start=first, stop=last)
                            nc.tensor.matmul(
                                o1, lhsT=sl, rhs=vt[:, jt, 512:1024],
                                start=first, stop=last)
                            nc.tensor.matmul(
                                rsx, lhsT=sl, rhs=ones16,
                                start=first, stop=last)

                    # scale + drain on DVE (Act stays clear for the next
                    # group's exp chain); low block first - its PSUM banks
                    # are the ones the next group needs soonest. The final
                    # block splits its two scalings across DVE and Act.
                    final = gi == len(groups) - 1
                    for (itx, o0, o1, rsx) in obanks:
                        ri = accp.tile([P, 1], F32, tag="ri")
                        nc.vector.reciprocal(ri, rsx)
                        for es, op in enumerate((o0, o1)):
                            ob = obuf.tile([P, 512], F32, tag="ob")
                            if final and es == 1:
                                nc.scalar.mul(ob, op, ri)
                            else:
                                nc.vector.tensor_scalar_mul(ob, op, ri)
                            nc.sync.dma_start(
                                out[itx * P:(itx + 1) * P,
                                    es * 512:(es + 1) * 512], ob)

    nc.finalize()
    return nc


_NC = None


def _get_nc():
    global _NC
    if _NC is None:
        _NC = build_graph()
    return _NC


def _run(inputs, trace=False, **kwargs):
    x = np.ascontiguousarray(np.asarray(inputs["input"], dtype=np.float32))
    k = np.ascontiguousarray(np.asarray(inputs["k"], dtype=np.float32))
    q = np.ascontiguousarray(np.asarray(inputs["q"], dtype=np.float32))
    v = np.ascontiguousarray(np.asarray(inputs["v"], dtype=np.float32))
    assert x.shape == (B, N, H)
    nc = _get_nc()
    in_maps = [
        {"input": x[b], "k": k, "q": q, "v": v} for b in range(B)
    ]
    res = bass_utils.run_bass_kernel_spmd(
        nc, in_maps, core_ids=list(range(B)), trace=trace, **kwargs)
    outs = np.stack([np.asarray(r["out"]) for r in res.results], axis=0)
    return outs.astype(np.float32), res


def kernel(**inputs):
    outs, _ = _run(inputs, trace=False)
    return outs
